# revision 1
# baseline (speedup 1.0000x reference)
"""Trainium2 Bass kernel for nn_DenseOnUp (gnn_message_passing).

Math: reference computes
    up = elu_mlp(x[sel])            # gather
    out = x + scatter_add(sel, up)  # scatter back to the SAME indices
Since the scatter indices equal the gather indices, duplicate selections
contribute identical MLP values, so
    out[n] = x[n] + count(n) * elu_mlp(x[n])   for selected n, x[n] otherwise.

Strategy (8 cores, data-parallel over rows):
  host:   bincount sel_idx; per core (50000 rows) and per quarter (12500 rows)
          build the sorted unique selected local indices + their counts,
          padded to a fixed size. Gather pads point at row 0 and scatter pads
          at a scratch output row with count 0, so pads contribute exactly
          zero and every index is valid (num_idxs is uniform across cores).
          Fold the elu "-1" into the next layer's bias: with
          h' = relu(z) + min(exp(z), 1) = elu(z) + 1,
          z_next = (h' - 1) @ W + b = h' @ W + (b - colsum(W)).
  device: dense-copy x -> out (the unavoidable memory floor), per-128-row
          indirect-DMA gathers of the unique selected rows (all quarters up
          front so the in-order gpsimd queue never stalls on the copy),
          transpose to feature-major, 3-layer MLP in fp32r (full PE rate at
          free dim 512), y = x + cnt*elu on DVE, transpose back, plain
          per-128-row indirect scatter writes into out after its dense copy
          (CCE scatter-add is ~2 GB/s read-modify-write on HW - avoid).
"""

import numpy as np

import concourse.bacc as bacc
import concourse.bass as bass
import concourse.mybir as mybir
import concourse.tile as tile
from concourse.bass_utils import run_bass_kernel_spmd
from concourse.masks import make_identity
from concourse.tile import add_dep_helper

F32 = mybir.dt.float32
F32R = mybir.dt.float32r
I32 = mybir.dt.int32
AF = mybir.ActivationFunctionType
OP = mybir.AluOpType

N_CORES = 8
N = 400000
F = 128
H = 256
RPC = N // N_CORES          # rows per core = 50000
NQ = 4                      # output slices per core (copy/scatter pipelining)
RPQ = RPC // NQ             # rows per quarter = 12500
OPAD = 32                   # scratch rows appended to each output slice
BLK = 512                   # MLP block rows (fp32r needs moving dim >= 256)
CPQ = 4                     # dense-copy chunks per quarter
CH = RPQ * F // 128 // CPQ  # free elems per copy chunk per partition

LAST_RUN_RESULTS = None     # BassKernelResults of the most recent run
LAST_NC = None              # compiled Bacc program of the most recent run
LAST_IN_MAPS = None         # per-core input maps of the most recent run


def _build_program(U_q):
    """Build the SPMD single-core program. U_q: padded unique rows per
    quarter (multiple of BLK)."""
    TPQ = U_q // 128            # gather/scatter 128-row tiles per quarter
    NT = NQ * TPQ               # total tiles per core
    BPQ = U_q // BLK            # MLP blocks per quarter

    nc = bacc.Bacc("TRN2", target_bir_lowering=False, debug=False,
                   num_devices=N_CORES)

    xk = nc.dram_tensor("x", [RPC, F], F32, kind="ExternalInput").ap()
    # int32 indices, [128, NT] with tile t / partition p at [p, t]:
    # gather uses core-local row ids (pads -> row 0), scatter quarter-local
    # ones (pads -> scratch row RPQ, which receives exact zeros)
    uidx_g = nc.dram_tensor("uidx_g", [128, NT], I32,
                            kind="ExternalInput").ap()
    uidx_s = nc.dram_tensor("uidx_s", [128, NT], I32,
                            kind="ExternalInput").ap()
    cntv = nc.dram_tensor("cnt", [128, NT], F32, kind="ExternalInput").ap()
    W0 = nc.dram_tensor("W0", [F, H], F32, kind="ExternalInput").ap()
    W1 = nc.dram_tensor("W1", [H, H], F32, kind="ExternalInput").ap()
    W2 = nc.dram_tensor("W2", [H, F], F32, kind="ExternalInput").ap()
    b0 = nc.dram_tensor("b0", [H], F32, kind="ExternalInput").ap()
    b1 = nc.dram_tensor("b1", [H], F32, kind="ExternalInput").ap()
    b2 = nc.dram_tensor("b2", [F], F32, kind="ExternalInput").ap()
    outs = [nc.dram_tensor(f"o{q}", [RPQ + OPAD, F], F32,
                           kind="ExternalOutput").ap()
            for q in range(NQ)]

    with tile.TileContext(nc) as tc:
        with (
            tc.tile_pool(name="const", bufs=1) as cpool,
            tc.tile_pool(name="copy", bufs=2) as copool,
            tc.tile_pool(name="gath", bufs=1) as gpool,
            tc.tile_pool(name="ytmp", bufs=2) as tpool,
            tc.tile_pool(name="act", bufs=2) as apool,
            tc.tile_pool(name="psum", bufs=1, space="PSUM") as ppool,
        ):
            ident = cpool.tile([128, 128], F32, tag="ident")
            make_identity(nc, ident[:])

            # weights as stationary lhsT tiles [K=128, M=128]; matmuls run in
            # fp32r, whose operands must be produced rounded -> copy via DVE
            def load_weight(tag, src_ap):
                tmp = cpool.tile([128, 128], F32, tag=tag + "_raw")
                nc.sync.dma_start(out=tmp[:], in_=src_ap)
                w = cpool.tile([128, 128], F32R, tag=tag)
                nc.vector.tensor_copy(out=w[:], in_=tmp[:])
                return w

            w0t = [load_weight(f"w0_{m}", W0[:, m * 128:(m + 1) * 128])
                   for m in range(2)]
            w1t = {(k, m): load_weight(
                       f"w1_{k}{m}",
                       W1[k * 128:(k + 1) * 128, m * 128:(m + 1) * 128])
                   for k in range(2) for m in range(2)}
            w2t = [load_weight(f"w2_{k}", W2[k * 128:(k + 1) * 128, :])
                   for k in range(2)]

            b0t, b1t = [], []
            for m in range(2):
                b = cpool.tile([128, 1], F32, tag=f"b0_{m}")
                nc.sync.dma_start(out=b[:], in_=b0[m * 128:(m + 1) * 128, None])
                b0t.append(b)
                b = cpool.tile([128, 1], F32, tag=f"b1_{m}")
                nc.sync.dma_start(out=b[:], in_=b1[m * 128:(m + 1) * 128, None])
                b1t.append(b)
            b2t = cpool.tile([128, 1], F32, tag="b2")
            nc.sync.dma_start(out=b2t[:], in_=b2[:, None])

            idxg_sb = cpool.tile([128, NT], I32, tag="idxg")
            nc.sync.dma_start(out=idxg_sb[:], in_=uidx_g[:])
            idxs_sb = cpool.tile([128, NT], I32, tag="idxs")
            nc.sync.dma_start(out=idxs_sb[:], in_=uidx_s[:])
            cnt_sb = cpool.tile([128, NT], F32, tag="cnt")
            nc.sync.dma_start(out=cnt_sb[:], in_=cntv[:])

            # ---- dense copies first: they gate the scatters ----
            copy_insts = {}
            for q in range(NQ):
                xq_flat = xk[q * RPQ:(q + 1) * RPQ, :].rearrange(
                    "a b -> (a b)").rearrange("(p c) -> p c", p=128)
                oq_flat = outs[q][0:RPQ, :].rearrange(
                    "a b -> (a b)").rearrange("(p c) -> p c", p=128)
                cis = []
                for j in range(CPQ):
                    ct = copool.tile([128, CH], F32, tag="cchunk")
                    nc.sync.dma_start(
                        out=ct[:], in_=xq_flat[:, j * CH:(j + 1) * CH])
                    ci = nc.sync.dma_start(
                        out=oq_flat[:, j * CH:(j + 1) * CH], in_=ct[:])
                    cis.append(ci)
                copy_insts[q] = cis

            def emit_gather(q):
                xg_q = gpool.tile([128, U_q], F32, tag=f"xgq{q % 2}")
                for t in range(TPQ):
                    nc.gpsimd.indirect_dma_start(
                        out=xg_q[:, t * 128:(t + 1) * 128], out_offset=None,
                        in_=xk[:],
                        in_offset=bass.IndirectOffsetOnAxis(
                            ap=idxg_sb[:, q * TPQ + t:q * TPQ + t + 1],
                            axis=0))
                return xg_q

            def emit_scatter(q, y_q):
                for t in range(TPQ):
                    s = nc.gpsimd.indirect_dma_start(
                        out=outs[q][:],
                        out_offset=bass.IndirectOffsetOnAxis(
                            ap=idxs_sb[:, q * TPQ + t:q * TPQ + t + 1],
                            axis=0),
                        in_=y_q[:, t * 128:(t + 1) * 128], in_offset=None)
                    for ci in copy_insts[q]:
                        add_dep_helper(s.ins, ci.ins,
                                       reason="scatter after dense copy")

            def emit_mlp(q, xg_q):
                y_q = gpool.tile([128, U_q], F32, tag=f"yq{q % 2}")
                for blk in range(BPQ):
                    c0 = blk * (BLK // 128)
                    t_in = ppool.tile([128, BLK], F32, tag="t_in")
                    for j in range(BLK // 128):
                        nc.tensor.transpose(
                            out=t_in[:, j * 128:(j + 1) * 128],
                            in_=xg_q[:, (c0 + j) * 128:(c0 + j + 1) * 128],
                            identity=ident[:])
                    a0 = apool.tile([128, BLK], F32R, tag="a0")
                    nc.vector.tensor_copy(out=a0[:], in_=t_in[:])

                    # L0: z0 = a0 @ W0 + b0  (features on partitions)
                    r0, w0 = [], []
                    for m in range(2):
                        p = ppool.tile([128, BLK], F32, tag=f"p0_{m}")
                        nc.tensor.matmul(
                            out=p[:], lhsT=w0t[m][:], rhs=a0[:],
                            start=True, stop=True)
                        r = apool.tile([128, BLK], F32R, tag=f"r0_{m}")
                        nc.scalar.activation(r[:], p[:], AF.Relu,
                                             bias=b0t[m][:, :1])
                        e = apool.tile([128, BLK], F32, tag=f"e0_{m}")
                        nc.scalar.activation(e[:], p[:], AF.Exp,
                                             bias=b0t[m][:, :1])
                        w_ = apool.tile([128, BLK], F32R, tag=f"w0c_{m}")
                        nc.vector.tensor_scalar(
                            out=w_[:], in0=e[:], scalar1=1.0, scalar2=None,
                            op0=OP.min)
                        r0.append(r)
                        w0.append(w_)

                    # L1: z1 = (r0 + w0) @ W1 + b1'
                    r1, w1 = [], []
                    for m in range(2):
                        p = ppool.tile([128, BLK], F32, tag=f"p1_{m}")
                        ops = [(w1t[(0, m)], r0[0]), (w1t[(0, m)], w0[0]),
                               (w1t[(1, m)], r0[1]), (w1t[(1, m)], w0[1])]
                        for i, (wt, act) in enumerate(ops):
                            nc.tensor.matmul(
                                out=p[:], lhsT=wt[:], rhs=act[:],
                                start=(i == 0), stop=(i == len(ops) - 1))
                        r = apool.tile([128, BLK], F32R, tag=f"r1_{m}")
                        nc.vector.tensor_scalar(
                            out=r[:], in0=p[:], scalar1=b1t[m][:, :1],
                            scalar2=0.0, op0=OP.add, op1=OP.max)
                        e = apool.tile([128, BLK], F32, tag=f"e1_{m}")
                        nc.scalar.activation(e[:], p[:], AF.Exp,
                                             bias=b1t[m][:, :1])
                        w_ = apool.tile([128, BLK], F32R, tag=f"w1c_{m}")
                        nc.vector.tensor_scalar(
                            out=w_[:], in0=e[:], scalar1=1.0, scalar2=None,
                            op0=OP.min)
                        r1.append(r)
                        w1.append(w_)

                    # L2: z2 = (r1 + w1) @ W2 + b2'
                    p2 = ppool.tile([128, BLK], F32, tag="p2")
                    ops = [(w2t[0], r1[0]), (w2t[0], w1[0]),
                           (w2t[1], r1[1]), (w2t[1], w1[1])]
                    for i, (wt, act) in enumerate(ops):
                        nc.tensor.matmul(
                            out=p2[:], lhsT=wt[:], rhs=act[:],
                            start=(i == 0), stop=(i == len(ops) - 1))
                    r2 = apool.tile([128, BLK], F32, tag="r2")
                    nc.scalar.activation(r2[:], p2[:], AF.Relu,
                                         bias=b2t[:, :1])
                    e2 = apool.tile([128, BLK], F32, tag="e2")
                    nc.scalar.activation(e2[:], p2[:], AF.Exp, bias=b2t[:, :1])
                    w2_ = apool.tile([128, BLK], F32, tag="w2c")
                    nc.vector.tensor_scalar(
                        out=w2_[:], in0=e2[:], scalar1=1.0, scalar2=None,
                        op0=OP.min)
                    h2 = apool.tile([128, BLK], F32, tag="h2")
                    nc.vector.tensor_tensor(
                        out=h2[:], in0=r2[:], in1=w2_[:], op=OP.add)

                    # back to row-major; y = xg + (cnt*h2' - cnt) = x + cnt*elu
                    t_out = ppool.tile([128, BLK], F32, tag="t_out")
                    for j in range(BLK // 128):
                        nc.tensor.transpose(
                            out=t_out[:, j * 128:(j + 1) * 128],
                            in_=h2[:, j * 128:(j + 1) * 128],
                            identity=ident[:])
                    for j in range(BLK // 128):
                        t = q * TPQ + c0 + j
                        ca = cnt_sb[:, t:t + 1]
                        yt = tpool.tile([128, F], F32, tag=f"yt{j}")
                        nc.vector.tensor_scalar(
                            out=yt[:],
                            in0=t_out[:, j * 128:(j + 1) * 128],
                            scalar1=ca, scalar2=ca,
                            op0=OP.mult, op1=OP.subtract)
                        nc.vector.tensor_tensor(
                            out=y_q[:, (c0 + j) * 128:(c0 + j + 1) * 128],
                            in0=yt[:],
                            in1=xg_q[:, (c0 + j) * 128:(c0 + j + 1) * 128],
                            op=OP.add)
                return y_q

            # pipeline: gathers stay ahead of scatters on the in-order
            # gpsimd queue; scatter q only needs copy q (done early)
            xg = {0: emit_gather(0), 1: emit_gather(1)}
            ys = {}
            for q in range(NQ):
                ys[q] = emit_mlp(q, xg[q])
                emit_scatter(q, ys[q])
                if q + 2 < NQ:
                    xg[q + 2] = emit_gather(q + 2)
    return nc


def _prep_host(x, sel_idx):
    """Per-core index-side inputs."""
    counts = np.bincount(sel_idx.reshape(-1), minlength=N)
    ulists, ucounts = {}, np.zeros((N_CORES, NQ), dtype=np.int64)
    for k in range(N_CORES):
        for q in range(NQ):
            lo = k * RPC + q * RPQ
            nz = np.nonzero(counts[lo:lo + RPQ])[0].astype(np.int32)
            ulists[(k, q)] = (nz, counts[lo + nz].astype(np.float32))
            ucounts[k, q] = len(nz)
    U_q = max(BLK, int(np.ceil(ucounts.max() / BLK)) * BLK)
    TPQ = U_q // 128

    idxg_maps, idxs_maps, cnt_maps = [], [], []
    for k in range(N_CORES):
        cols_g, cols_s, cols_c = [], [], []
        for q in range(NQ):
            nz, cv = ulists[(k, q)]
            ag = np.zeros(U_q, dtype=np.int32)          # gather pad -> row 0
            as_ = np.full(U_q, RPQ, dtype=np.int32)     # scatter pad -> scratch
            ac = np.zeros(U_q, dtype=np.float32)
            ag[:len(nz)] = nz + q * RPQ                 # core-local for gather
            as_[:len(nz)] = nz                          # quarter-local scatter
            ac[:len(cv)] = cv
            cols_g.append(ag.reshape(TPQ, 128).T)
            cols_s.append(as_.reshape(TPQ, 128).T)
            cols_c.append(ac.reshape(TPQ, 128).T)
        idxg_maps.append(np.ascontiguousarray(np.hstack(cols_g)))
        idxs_maps.append(np.ascontiguousarray(np.hstack(cols_s)))
        cnt_maps.append(np.ascontiguousarray(np.hstack(cols_c)))
    return U_q, idxg_maps, idxs_maps, cnt_maps


def kernel(x, sel_idx, W0, b0, W1, b1, W2, b2):
    x = np.ascontiguousarray(np.asarray(x, dtype=np.float32))
    sel_idx = np.asarray(sel_idx, dtype=np.int32)
    W0, W1, W2 = [np.ascontiguousarray(np.asarray(w, dtype=np.float32))
                  for w in (W0, W1, W2)]
    b0, b1, b2 = [np.asarray(b, dtype=np.float32) for b in (b0, b1, b2)]

    # fold elu's "-1" into the next layer's bias: inputs to L1/L2 are elu+1
    b1f = np.ascontiguousarray(b1 - W1.sum(axis=0))
    b2f = np.ascontiguousarray(b2 - W2.sum(axis=0))

    U_q, idxg_maps, idxs_maps, cnt_maps = _prep_host(x, sel_idx)
    nc = _build_program(U_q)
    nc.compile()

    in_maps = []
    for k in range(N_CORES):
        in_maps.append({
            "x": np.ascontiguousarray(x[k * RPC:(k + 1) * RPC]),
            "uidx_g": idxg_maps[k],
            "uidx_s": idxs_maps[k],
            "cnt": cnt_maps[k],
            "W0": W0, "W1": W1, "W2": W2,
            "b0": b0, "b1": b1f, "b2": b2f,
        })
    global LAST_RUN_RESULTS, LAST_NC, LAST_IN_MAPS
    LAST_NC, LAST_IN_MAPS = nc, in_maps
    res = run_bass_kernel_spmd(nc, in_maps, core_ids=list(range(N_CORES)))
    LAST_RUN_RESULTS = res

    out = np.empty_like(x)
    for k in range(N_CORES):
        for q in range(NQ):
            lo = k * RPC + q * RPQ
            out[lo:lo + RPQ] = res.results[k][f"o{q}"][:RPQ]
    return out



# revision 9
# speedup vs baseline: 31.3874x; 31.3874x over previous
"""Trainium2 Bass kernel for nn_DenseOnUp (gnn_message_passing).

Math: reference computes
    up = elu_mlp(x[sel])            # gather
    out = x + scatter_add(sel, up)  # scatter back to the SAME indices
Since the scatter indices equal the gather indices, duplicate selections
contribute identical MLP values, so
    out[n] = x[n] + count(n) * elu_mlp(x[n])   for selected n, x[n] otherwise.

Strategy (8 cores, data-parallel over rows):
  host:   bincount sel_idx; per core (50000 rows) and per quarter (12500 rows)
          build the sorted unique selected local indices + their counts,
          padded to a fixed size. Gather pads point at row 0 and scatter pads
          at a scratch output row with count 0, so pads contribute exactly
          zero and every index is valid (num_idxs is uniform across cores).
          Fold the elu "-1" into the next layer's bias: with
          h' = relu(z) + min(exp(z), 1) = elu(z) + 1,
          z_next = (h' - 1) @ W + b = h' @ W + (b - colsum(W)).
  device: dense-copy x -> out (the unavoidable memory floor), per-128-row
          indirect-DMA gathers of the unique selected rows (all quarters up
          front so the in-order gpsimd queue never stalls on the copy),
          transpose to feature-major, 3-layer MLP in fp32r (full PE rate at
          free dim 512), y = x + cnt*elu on DVE, transpose back, plain
          per-128-row indirect scatter writes into out after its dense copy
          (CCE scatter-add is ~2 GB/s read-modify-write on HW - avoid).
"""

import numpy as np

import concourse.bacc as bacc
import concourse.bass as bass
import concourse.mybir as mybir
import concourse.tile as tile
from concourse.bass_utils import run_bass_kernel_spmd
from concourse.masks import make_identity
from concourse.tile import add_dep_helper

F32 = mybir.dt.float32
F32R = mybir.dt.float32r
I32 = mybir.dt.int32
AF = mybir.ActivationFunctionType
OP = mybir.AluOpType

N_CORES = 8
N = 400000
F = 128
H = 256
RPC = N // N_CORES          # rows per core = 50000
NQ = 4                      # output slices per core (copy/scatter pipelining)
RPQ = RPC // NQ             # rows per quarter = 12500
OPAD = 32                   # scratch rows appended to each output slice
BLK = 512                   # MLP block rows (fp32r needs moving dim >= 256)
CPQ = 4                     # dense-copy chunks per quarter
CH = RPQ * F // 128 // CPQ  # free elems per copy chunk per partition

LAST_RUN_RESULTS = None     # BassKernelResults of the most recent run
LAST_NC = None              # compiled Bacc program of the most recent run
LAST_IN_MAPS = None         # per-core input maps of the most recent run
LAST_UQ = None              # U_q of the most recent run


def _build_program(U_q, reps=1):
    """Build the SPMD single-core program. U_q: padded unique rows per
    quarter (multiple of BLK). reps>1 repeats the whole body sequentially
    (used only for device-time measurement; final output identical)."""
    TPQ = U_q // 128            # gather/scatter 128-row tiles per quarter
    NT = NQ * TPQ               # total tiles per core
    BPQ = U_q // BLK            # MLP blocks per quarter

    nc = bacc.Bacc("TRN2", target_bir_lowering=False, debug=False,
                   num_devices=N_CORES)

    xk = nc.dram_tensor("x", [RPC, F], F32, kind="ExternalInput").ap()
    # int32 indices, [128, NT] with tile t / partition p at [p, t]:
    # gather uses core-local row ids (pads -> row 0), scatter quarter-local
    # ones (pads -> scratch row RPQ, which receives exact zeros)
    uidx_g = nc.dram_tensor("uidx_g", [128, NT], I32,
                            kind="ExternalInput").ap()
    uidx_s = nc.dram_tensor("uidx_s", [128, NT], I32,
                            kind="ExternalInput").ap()
    cntv = nc.dram_tensor("cnt", [128, NT], F32, kind="ExternalInput").ap()
    W0 = nc.dram_tensor("W0", [F, H], F32, kind="ExternalInput").ap()
    W1 = nc.dram_tensor("W1", [H, H], F32, kind="ExternalInput").ap()
    W2 = nc.dram_tensor("W2", [H, F], F32, kind="ExternalInput").ap()
    b0 = nc.dram_tensor("b0", [H], F32, kind="ExternalInput").ap()
    b1 = nc.dram_tensor("b1", [H], F32, kind="ExternalInput").ap()
    b2 = nc.dram_tensor("b2", [F], F32, kind="ExternalInput").ap()
    outs = [nc.dram_tensor(f"o{q}", [RPQ + OPAD, F], F32,
                           kind="ExternalOutput").ap()
            for q in range(NQ)]

    with tile.TileContext(nc) as tc:
        with (
            tc.tile_pool(name="const", bufs=1) as cpool,
            tc.tile_pool(name="copy", bufs=2) as copool,
            tc.tile_pool(name="gath", bufs=1) as gpool,
            tc.tile_pool(name="ytmp", bufs=2) as tpool,
            tc.tile_pool(name="act", bufs=2) as apool,
            tc.tile_pool(name="psum", bufs=1, space="PSUM") as ppool,
        ):
            ident = cpool.tile([128, 128], F32, tag="ident")
            make_identity(nc, ident[:])

            # weights as stationary lhsT tiles [K=128, M=128]; matmuls run in
            # fp32r, whose operands must be produced rounded -> copy via DVE
            def load_weight(tag, src_ap):
                tmp = cpool.tile([128, 128], F32, tag=tag + "_raw")
                nc.sync.dma_start(out=tmp[:], in_=src_ap)
                w = cpool.tile([128, 128], F32R, tag=tag)
                nc.vector.tensor_copy(out=w[:], in_=tmp[:])
                return w

            w0t = [load_weight(f"w0_{m}", W0[:, m * 128:(m + 1) * 128])
                   for m in range(2)]
            w1t = {(k, m): load_weight(
                       f"w1_{k}{m}",
                       W1[k * 128:(k + 1) * 128, m * 128:(m + 1) * 128])
                   for k in range(2) for m in range(2)}
            w2t = [load_weight(f"w2_{k}", W2[k * 128:(k + 1) * 128, :])
                   for k in range(2)]

            b0t, b1t = [], []
            for m in range(2):
                b = cpool.tile([128, 1], F32, tag=f"b0_{m}")
                nc.sync.dma_start(out=b[:], in_=b0[m * 128:(m + 1) * 128, None])
                b0t.append(b)
                b = cpool.tile([128, 1], F32, tag=f"b1_{m}")
                nc.sync.dma_start(out=b[:], in_=b1[m * 128:(m + 1) * 128, None])
                b1t.append(b)
            b2t = cpool.tile([128, 1], F32, tag="b2")
            nc.sync.dma_start(out=b2t[:], in_=b2[:, None])

            idxg_sb = cpool.tile([128, NT], I32, tag="idxg")
            nc.sync.dma_start(out=idxg_sb[:], in_=uidx_g[:])
            idxs_sb = cpool.tile([128, NT], I32, tag="idxs")
            nc.sync.dma_start(out=idxs_sb[:], in_=uidx_s[:])
            cnt_sb = cpool.tile([128, NT], F32, tag="cnt")
            nc.sync.dma_start(out=cnt_sb[:], in_=cntv[:])

            # ---- dense copies first: they gate the scatters ----
            def emit_copies():
                copy_insts = {}
                for q in range(NQ):
                    xq_flat = xk[q * RPQ:(q + 1) * RPQ, :].rearrange(
                        "a b -> (a b)").rearrange("(p c) -> p c", p=128)
                    oq_flat = outs[q][0:RPQ, :].rearrange(
                        "a b -> (a b)").rearrange("(p c) -> p c", p=128)
                    cis = []
                    for j in range(CPQ):
                        ct = copool.tile([128, CH], F32, tag="cchunk")
                        nc.sync.dma_start(
                            out=ct[:], in_=xq_flat[:, j * CH:(j + 1) * CH])
                        ci = nc.sync.dma_start(
                            out=oq_flat[:, j * CH:(j + 1) * CH], in_=ct[:])
                        cis.append(ci)
                    copy_insts[q] = cis
                return copy_insts

            def emit_gather(q):
                xg_q = gpool.tile([128, U_q], F32, tag=f"xgq{q % 2}")
                for t in range(TPQ):
                    nc.gpsimd.indirect_dma_start(
                        out=xg_q[:, t * 128:(t + 1) * 128], out_offset=None,
                        in_=xk[:],
                        in_offset=bass.IndirectOffsetOnAxis(
                            ap=idxg_sb[:, q * TPQ + t:q * TPQ + t + 1],
                            axis=0))
                return xg_q

            def emit_scatter(q, y_q, copy_insts):
                for t in range(TPQ):
                    s = nc.gpsimd.indirect_dma_start(
                        out=outs[q][:],
                        out_offset=bass.IndirectOffsetOnAxis(
                            ap=idxs_sb[:, q * TPQ + t:q * TPQ + t + 1],
                            axis=0),
                        in_=y_q[:, t * 128:(t + 1) * 128], in_offset=None)
                    for ci in copy_insts[q]:
                        add_dep_helper(s.ins, ci.ins,
                                       reason="scatter after dense copy")

            def emit_mlp(q, xg_q):
                y_q = gpool.tile([128, U_q], F32, tag=f"yq{q % 2}")
                for blk in range(BPQ):
                    c0 = blk * (BLK // 128)
                    t_in = ppool.tile([128, BLK], F32, tag="t_in")
                    for j in range(BLK // 128):
                        nc.tensor.transpose(
                            out=t_in[:, j * 128:(j + 1) * 128],
                            in_=xg_q[:, (c0 + j) * 128:(c0 + j + 1) * 128],
                            identity=ident[:])
                    a0 = apool.tile([128, BLK], F32R, tag="a0")
                    nc.vector.tensor_copy(out=a0[:], in_=t_in[:])

                    # L0: z0 = a0 @ W0 + b0  (features on partitions)
                    r0, w0 = [], []
                    for m in range(2):
                        p = ppool.tile([128, BLK], F32, tag=f"p0_{m}")
                        nc.tensor.matmul(
                            out=p[:], lhsT=w0t[m][:], rhs=a0[:],
                            start=True, stop=True)
                        r = apool.tile([128, BLK], F32R, tag=f"r0_{m}")
                        nc.scalar.activation(r[:], p[:], AF.Relu,
                                             bias=b0t[m][:, :1])
                        e = apool.tile([128, BLK], F32, tag=f"e0_{m}")
                        nc.scalar.activation(e[:], p[:], AF.Exp,
                                             bias=b0t[m][:, :1])
                        w_ = apool.tile([128, BLK], F32R, tag=f"w0c_{m}")
                        nc.vector.tensor_scalar(
                            out=w_[:], in0=e[:], scalar1=1.0, scalar2=None,
                            op0=OP.min)
                        r0.append(r)
                        w0.append(w_)

                    # L1: z1 = (r0 + w0) @ W1 + b1'
                    r1, w1 = [], []
                    for m in range(2):
                        p = ppool.tile([128, BLK], F32, tag=f"p1_{m}")
                        ops = [(w1t[(0, m)], r0[0]), (w1t[(0, m)], w0[0]),
                               (w1t[(1, m)], r0[1]), (w1t[(1, m)], w0[1])]
                        for i, (wt, act) in enumerate(ops):
                            nc.tensor.matmul(
                                out=p[:], lhsT=wt[:], rhs=act[:],
                                start=(i == 0), stop=(i == len(ops) - 1))
                        r = apool.tile([128, BLK], F32R, tag=f"r1_{m}")
                        nc.vector.tensor_scalar(
                            out=r[:], in0=p[:], scalar1=b1t[m][:, :1],
                            scalar2=0.0, op0=OP.add, op1=OP.max)
                        e = apool.tile([128, BLK], F32, tag=f"e1_{m}")
                        nc.scalar.activation(e[:], p[:], AF.Exp,
                                             bias=b1t[m][:, :1])
                        w_ = apool.tile([128, BLK], F32R, tag=f"w1c_{m}")
                        nc.vector.tensor_scalar(
                            out=w_[:], in0=e[:], scalar1=1.0, scalar2=None,
                            op0=OP.min)
                        r1.append(r)
                        w1.append(w_)

                    # L2: z2 = (r1 + w1) @ W2 + b2'
                    p2 = ppool.tile([128, BLK], F32, tag="p2")
                    ops = [(w2t[0], r1[0]), (w2t[0], w1[0]),
                           (w2t[1], r1[1]), (w2t[1], w1[1])]
                    for i, (wt, act) in enumerate(ops):
                        nc.tensor.matmul(
                            out=p2[:], lhsT=wt[:], rhs=act[:],
                            start=(i == 0), stop=(i == len(ops) - 1))
                    r2 = apool.tile([128, BLK], F32, tag="r2")
                    nc.scalar.activation(r2[:], p2[:], AF.Relu,
                                         bias=b2t[:, :1])
                    e2 = apool.tile([128, BLK], F32, tag="e2")
                    nc.scalar.activation(e2[:], p2[:], AF.Exp, bias=b2t[:, :1])
                    w2_ = apool.tile([128, BLK], F32, tag="w2c")
                    nc.vector.tensor_scalar(
                        out=w2_[:], in0=e2[:], scalar1=1.0, scalar2=None,
                        op0=OP.min)
                    h2 = apool.tile([128, BLK], F32, tag="h2")
                    nc.vector.tensor_tensor(
                        out=h2[:], in0=r2[:], in1=w2_[:], op=OP.add)

                    # back to row-major; y = xg + (cnt*h2' - cnt) = x + cnt*elu
                    t_out = ppool.tile([128, BLK], F32, tag="t_out")
                    for j in range(BLK // 128):
                        nc.tensor.transpose(
                            out=t_out[:, j * 128:(j + 1) * 128],
                            in_=h2[:, j * 128:(j + 1) * 128],
                            identity=ident[:])
                    for j in range(BLK // 128):
                        t = q * TPQ + c0 + j
                        ca = cnt_sb[:, t:t + 1]
                        yt = tpool.tile([128, F], F32, tag=f"yt{j}")
                        nc.vector.tensor_scalar(
                            out=yt[:],
                            in0=t_out[:, j * 128:(j + 1) * 128],
                            scalar1=ca, scalar2=ca,
                            op0=OP.mult, op1=OP.subtract)
                        nc.vector.tensor_tensor(
                            out=y_q[:, (c0 + j) * 128:(c0 + j + 1) * 128],
                            in0=yt[:],
                            in1=xg_q[:, (c0 + j) * 128:(c0 + j + 1) * 128],
                            op=OP.add)
                return y_q

            # pipeline: gathers stay ahead of scatters on the in-order
            # gpsimd queue; scatter q only needs copy q (done early)
            for _rep in range(reps):
                copy_insts = emit_copies()
                xg = {0: emit_gather(0), 1: emit_gather(1)}
                ys = {}
                for q in range(NQ):
                    ys[q] = emit_mlp(q, xg[q])
                    emit_scatter(q, ys[q], copy_insts)
                    if q + 2 < NQ:
                        xg[q + 2] = emit_gather(q + 2)
    return nc


def _prep_host(x, sel_idx):
    """Per-core index-side inputs."""
    counts = np.bincount(sel_idx.reshape(-1), minlength=N)
    ulists, ucounts = {}, np.zeros((N_CORES, NQ), dtype=np.int64)
    for k in range(N_CORES):
        for q in range(NQ):
            lo = k * RPC + q * RPQ
            nz = np.nonzero(counts[lo:lo + RPQ])[0].astype(np.int32)
            ulists[(k, q)] = (nz, counts[lo + nz].astype(np.float32))
            ucounts[k, q] = len(nz)
    U_q = max(BLK, int(np.ceil(ucounts.max() / BLK)) * BLK)
    TPQ = U_q // 128

    idxg_maps, idxs_maps, cnt_maps = [], [], []
    for k in range(N_CORES):
        cols_g, cols_s, cols_c = [], [], []
        for q in range(NQ):
            nz, cv = ulists[(k, q)]
            ag = np.zeros(U_q, dtype=np.int32)          # gather pad -> row 0
            as_ = np.full(U_q, RPQ, dtype=np.int32)     # scatter pad -> scratch
            ac = np.zeros(U_q, dtype=np.float32)
            ag[:len(nz)] = nz + q * RPQ                 # core-local for gather
            as_[:len(nz)] = nz                          # quarter-local scatter
            ac[:len(cv)] = cv
            cols_g.append(ag.reshape(TPQ, 128).T)
            cols_s.append(as_.reshape(TPQ, 128).T)
            cols_c.append(ac.reshape(TPQ, 128).T)
        idxg_maps.append(np.ascontiguousarray(np.hstack(cols_g)))
        idxs_maps.append(np.ascontiguousarray(np.hstack(cols_s)))
        cnt_maps.append(np.ascontiguousarray(np.hstack(cols_c)))
    return U_q, idxg_maps, idxs_maps, cnt_maps


def kernel(x, sel_idx, W0, b0, W1, b1, W2, b2):
    x = np.ascontiguousarray(np.asarray(x, dtype=np.float32))
    sel_idx = np.asarray(sel_idx, dtype=np.int32)
    W0, W1, W2 = [np.ascontiguousarray(np.asarray(w, dtype=np.float32))
                  for w in (W0, W1, W2)]
    b0, b1, b2 = [np.asarray(b, dtype=np.float32) for b in (b0, b1, b2)]

    # fold elu's "-1" into the next layer's bias: inputs to L1/L2 are elu+1
    b1f = np.ascontiguousarray(b1 - W1.sum(axis=0))
    b2f = np.ascontiguousarray(b2 - W2.sum(axis=0))

    U_q, idxg_maps, idxs_maps, cnt_maps = _prep_host(x, sel_idx)
    nc = _build_program(U_q)
    nc.compile()

    in_maps = []
    for k in range(N_CORES):
        in_maps.append({
            "x": np.ascontiguousarray(x[k * RPC:(k + 1) * RPC]),
            "uidx_g": idxg_maps[k],
            "uidx_s": idxs_maps[k],
            "cnt": cnt_maps[k],
            "W0": W0, "W1": W1, "W2": W2,
            "b0": b0, "b1": b1f, "b2": b2f,
        })
    global LAST_RUN_RESULTS, LAST_NC, LAST_IN_MAPS, LAST_UQ
    LAST_NC, LAST_IN_MAPS, LAST_UQ = nc, in_maps, U_q
    res = run_bass_kernel_spmd(nc, in_maps, core_ids=list(range(N_CORES)))
    LAST_RUN_RESULTS = res

    out = np.empty_like(x)
    for k in range(N_CORES):
        for q in range(NQ):
            lo = k * RPC + q * RPQ
            out[lo:lo + RPQ] = res.results[k][f"o{q}"][:RPQ]
    return out

